# revision 1
# baseline (speedup 1.0000x reference)
"""Mixture-of-Softmax loss kernel for 8 Trainium2 NeuronCores.

out[s,v] = logsumexp_k( log_softmax_v(logits[s,k,v]) + log pi[s,k] )
         = log( sum_k pi[s,k] * exp(logits[s,k,v]) / Z[s,k] )

Sharding: vocab dimension of weight_matrix split across 8 cores (V=50257
padded to 50264 = 8*6283 with zero rows; the 7 pad columns contribute
exactly exp(0)=1 to the last core's local sum-of-exp and are subtracted
out via a per-core correction input, then dropped on gather).

Per core, per 128-token s-tile:
  PE   : logits[k] = projT[k]^T @ WT    (bf16, fp32 PSUM accumulate)
  ACT  : E = exp(logits) (fp16 in SBUF) with accum_out = per-chunk sums
  CC   : AllReduce(add) of local [128,2] sum-of-exp -> global Z
  DVE  : w_k = pi_k / Z_k ;  t = E0*(w0/w1) + E1
  ACT  : out = Ln(t * w1)
Logits are small (|l| < ~3 for this input distribution) so no max
subtraction is needed for a stable sum-of-exp in fp32.
"""

import math
import os
import sys

import numpy as np

for _p in ("/opt/trn_rl_repo", "/opt/trn_rl_repo/concourse"):
    if os.path.isdir(_p) and _p not in sys.path:
        sys.path.insert(0, _p)

import ml_dtypes

import concourse.bacc as bacc
import concourse.hw_specs as hw_specs
import concourse.tile as tile
from concourse import mybir
from concourse.bass_utils import run_bass_kernel_spmd

# --- Activation-table patch -------------------------------------------------
# This kernel interleaves Exp (sum-of-exp pass) and Ln (output pass) on the
# scalar engine. The default table chooser assigns Exp -> "exp_and_others"
# and Ln -> "natural_log", causing a ~2.7us ACT_TABLE_LOAD on every switch
# (hundreds of switches => ~0.8ms). The "natural_log_exp_and_others" set
# contains BOTH functions; hide Exp/Ln from every other set so the chooser
# must use the combined set, making the table resident for the whole kernel.
_orig_get_activation_tables = hw_specs.get_activation_tables


def _patched_get_activation_tables(module_arch):
    tabs = _orig_get_activation_tables(module_arch)
    E = mybir.ActivationFunctionType.Exp
    L = mybir.ActivationFunctionType.Ln
    out = {}
    for name, funcs in tabs.items():
        if name != "natural_log_exp_and_others" and (E in funcs or L in funcs):
            funcs = funcs - {E, L}
        out[name] = funcs
    return out


bacc.get_activation_tables = _patched_get_activation_tables
# ---------------------------------------------------------------------------

BF16 = mybir.dt.bfloat16
FP16 = mybir.dt.float16
FP32 = mybir.dt.float32
P = 128  # partitions


def _ceil_div(a, b):
    return (a + b - 1) // b


def build_program(n_cores=8, S=2048, D=1024, VS=6283, KM=2, e_dtype=FP16,
                  use_collectives=True, reps=1, ln_func=None):
    """Build the SPMD Bass program (same program on all cores).

    Inputs (per core):
      hiddenT  [D, S]   bf16   (same on all cores)
      w_projT  [D, KM*D] bf16  (same on all cores)
      w_gateT  [D, KM]  bf16   (same on all cores)
      wt       [D, VS]  bf16   (core's vocab shard of weight_matrix^T)
      corr     [P, 1]   f32    (number of pad columns in this core's shard)
    Output (per core):
      out      [S, VS]  f32
    """
    DC = D // P           # contraction chunks
    ST = S // P           # token tiles
    J = KM * D
    JT = J // P           # projT row tiles
    VCHUNK = 512
    vchunks = []
    v0 = 0
    while v0 < VS:
        w = min(VCHUNK, VS - v0)
        vchunks.append((v0, w))
        v0 += w
    NVC = len(vchunks)
    RG = [list(range(n_cores))]
    if ln_func is None:
        ln_func = mybir.ActivationFunctionType.Ln

    nc = bacc.Bacc(
        "TRN2",
        target_bir_lowering=False,
        debug=False,
        num_devices=n_cores,
    )

    hiddenT = nc.dram_tensor("hiddenT", [D, S], BF16, kind="ExternalInput").ap()
    hiddenTs = nc.dram_tensor(
        "hiddenTs", [D, S // n_cores], BF16, kind="ExternalInput"
    ).ap()
    w_projT = nc.dram_tensor("w_projT", [D, J], BF16, kind="ExternalInput").ap()
    w_gateT = nc.dram_tensor("w_gateT", [D, KM], BF16, kind="ExternalInput").ap()
    wt = nc.dram_tensor("wt", [D, VS], BF16, kind="ExternalInput").ap()
    corr = nc.dram_tensor("corr", [P, 1], FP32, kind="ExternalInput").ap()
    out = nc.dram_tensor("out", [S, VS], FP32, kind="ExternalOutput").ap()

    ht_r = hiddenT.rearrange("(c p) s -> c p s", p=P)
    hts_r = hiddenTs.rearrange("(c p) s -> c p s", p=P)
    wp_r = w_projT.rearrange("(c p) j -> c p j", p=P)
    wg_r = w_gateT.rearrange("(c p) k -> c p k", p=P)
    wt_r = wt.rearrange("(c p) v -> c p v", p=P)

    def emit_once(tc):
        with (
            tc.tile_pool(name="singles", bufs=1) as singles,
            tc.tile_pool(name="gates", bufs=ST) as gates,
            tc.tile_pool(name="dram", bufs=1, space="DRAM") as dpool,
            tc.tile_pool(name="pj", bufs=3) as pjp,
        ):
            PJ_PRELOAD = 3

            def load_pj(i):
                srow = i * P
                ci = srow // SSH
                soff = srow - ci * SSH
                PJ = pjp.tile([P, JT, P], BF16, tag="PJ", name=f"PJ_{i}")
                nc.sync.dma_start(
                    out=PJ,
                    in_=proj_ag[ci][:, :, soff:soff + P].rearrange(
                        "t p s -> p t s"
                    ),
                )
                return PJ

            # Resident vocab-shard weights [p, d-chunk, v], one tile per
            # v-chunk so the first matmuls only wait on their own slice of
            # the load, not the full 13MB.
            WTs = []
            for ci, (v0, w) in enumerate(vchunks):
                wt_tile = singles.tile([P, DC, w], BF16, tag=f"wt{ci}",
                                       name=f"WT_{ci}")
                for c in range(DC):
                    nc.sync.dma_start(out=wt_tile[:, c, :],
                                      in_=wt_r[c][:, v0:v0 + w])
                WTs.append(wt_tile)
            corr_sb = singles.tile([P, 1], FP32)
            nc.sync.dma_start(out=corr_sb, in_=corr)

            # Phase 0 is sharded over cores: each core computes projT for
            # S/n_cores tokens, then an AllGather replicates the full projT.
            # Results are bit-identical to local compute (same bf16 ops).
            SSH = S // n_cores  # tokens per core in phase 0
            assert SSH % P == 0 or n_cores == 1
            proj_in = dpool.tile([JT, P, SSH], BF16, name="proj_in")
            cc_addr = "Shared" if n_cores > 4 else "Local"
            proj_ag = dpool.tile([n_cores, JT, P, SSH], BF16, name="proj_ag",
                                 addr_space=cc_addr)
            ge_tiles = []
            rse_tiles = []

            # ACT-order chain (see comment at the main loop): order-only
            # edges keep the scalar engine's instruction stream in emission
            # order so Exp/Ln table swaps stay rare.
            last_act = [None]

            def act_chain(inst):
                if last_act[0] is not None:
                    tile.add_dep_helper(inst.ins, last_act[0].ins, sync=False,
                                        reason="act table batching")
                last_act[0] = inst
                return inst

            # ---------------- Phase 0: projT = (hidden @ w_proj^T)^T, gate ----
            with (
                tc.tile_pool(name="ph0", bufs=1) as ph0,
                tc.tile_pool(name="ph0ps", bufs=4, space="PSUM") as ps0,
                tc.tile_pool(name="ph0gps", bufs=2, space="PSUM") as gps0,
                tc.tile_pool(name="ph0st", bufs=4) as stg,
            ):
                HT = ph0.tile([P, DC, S], BF16)
                HTS = ph0.tile([P, DC, SSH], BF16)
                WP = ph0.tile([P, DC, J], BF16)
                WG = ph0.tile([P, DC, KM], BF16)
                for c in range(DC):
                    nc.sync.dma_start(out=HTS[:, c, :], in_=hts_r[c])
                    nc.sync.dma_start(out=WP[:, c, :], in_=wp_r[c])
                    nc.sync.dma_start(out=WG[:, c, :], in_=wg_r[c])
                    nc.sync.dma_start(out=HT[:, c, :], in_=ht_r[c])

                # projT[j, s] = sum_d w_projT[d, j] * hiddenT[d, s], for
                # this core's S/n_cores token slice; AllGather replicates.
                pj_tiles = {}
                PSC = min(512, SSH)
                for t in range(JT):
                    for s0 in range(0, SSH, PSC):
                        sw = min(PSC, SSH - s0)
                        psum = ps0.tile([P, PSC], FP32, tag="mm")
                        for d in range(DC):
                            nc.tensor.matmul(
                                psum[:, :sw],
                                lhsT=WP[:, d, t * P:(t + 1) * P],
                                rhs=HTS[:, d, s0:s0 + sw],
                                start=(d == 0),
                                stop=(d == DC - 1),
                            )
                        st = stg.tile([P, PSC], BF16, tag="st")
                        nc.vector.tensor_copy(st[:, :sw], psum[:, :sw])
                        nc.sync.dma_start(out=proj_in[t, :, s0:s0 + sw],
                                          in_=st[:, :sw])
                if use_collectives:
                    nc.gpsimd.collective_compute(
                        "AllGather",
                        mybir.AluOpType.bypass,
                        replica_groups=RG,
                        ins=[proj_in.opt()],
                        outs=[proj_ag.opt()],
                    )
                else:
                    nc.sync.dma_start(out=proj_ag[0], in_=proj_in[:])
                # Prefetch the first main-loop lhsT slices now so their
                # DMAs aren't queued behind the rest of phase 0.
                for i in range(min(PJ_PRELOAD, ST)):
                    pj_tiles[i] = load_pj(i)

                # gate logits -> pi (unnormalized e, and 1/sum_e)
                for i in range(ST):
                    gp = gps0.tile([P, KM], FP32, tag="g")
                    for d in range(DC):
                        nc.tensor.matmul(
                            gp,
                            lhsT=HT[:, d, i * P:(i + 1) * P],
                            rhs=WG[:, d, :],
                            start=(d == 0),
                            stop=(d == DC - 1),
                        )
                    negm = gates.tile([P, 1], FP32, tag="negm")
                    nc.vector.reduce_max(
                        out=negm, in_=gp, axis=mybir.AxisListType.X, negate=True
                    )
                    ge = gates.tile([P, KM], FP32, tag="ge")
                    se = gates.tile([P, 1], FP32, tag="se")
                    act_chain(nc.scalar.activation(
                        out=ge, in_=gp, func=mybir.ActivationFunctionType.Exp,
                        bias=negm, accum_out=se,
                    ))
                    rse = gates.tile([P, 1], FP32, tag="rse")
                    nc.vector.reciprocal(rse, se)
                    ge_tiles.append(ge)
                    rse_tiles.append(rse)

            # ---------------- Main loop over token tiles ----------------------
            with (
                tc.tile_pool(name="ebuf", bufs=2) as ep,
                tc.tile_pool(name="zp", bufs=2) as zpp,
                tc.tile_pool(name="mmps", bufs=7, space="PSUM") as psm,
                tc.tile_pool(name="ocp", bufs=6) as ocp,
                tc.tile_pool(name="ttp", bufs=6) as ttp,
                tc.tile_pool(name="s2", bufs=3) as s2p,
                tc.tile_pool(name="cc", bufs=2 * ST, space="DRAM") as ccp,
            ):
                # The scalar engine pays ~2.7us to swap activation tables
                # between Exp and Ln. The ACT chain keeps the stream in
                # emission order: [exp k0 (tile i)] [ln (tile i-1)]
                # [exp k1 (tile i)] -> 2 table swaps per s-tile instead of
                # O(chunks) swaps from priority-heap interleaving.
                def emit_exps(i, k, E, zpart, PJ):
                    for ci, (v0, w) in enumerate(vchunks):
                        ps = psm.tile([P, VCHUNK], FP32, tag="mm")
                        for d in range(DC):
                            nc.tensor.matmul(
                                ps[:, :w],
                                lhsT=PJ[:, k * DC + d, :],
                                rhs=WTs[ci][:, d, :w],
                                start=(d == 0),
                                stop=(d == DC - 1),
                            )
                        act_chain(nc.scalar.activation(
                            out=E[:, k, v0:v0 + w],
                            in_=ps[:, :w],
                            func=mybir.ActivationFunctionType.Exp,
                            accum_out=zpart[:, k, ci:ci + 1],
                        ))

                def emit_stage2(i, E, Zg):
                    srow = i * P
                    # w_k = pi_k / Z_k = ge_k * rse / Z_k
                    rz = s2p.tile([P, KM], FP32, tag="rz")
                    nc.vector.reciprocal(rz, Zg)
                    rzs = s2p.tile([P, KM], FP32, tag="rzs")
                    nc.vector.tensor_scalar_mul(rzs, rz, rse_tiles[i])
                    wk = s2p.tile([P, KM], FP32, tag="wk")
                    nc.vector.tensor_mul(wk, ge_tiles[i], rzs)
                    rw1 = s2p.tile([P, 1], FP32, tag="rw1")
                    nc.vector.reciprocal(rw1, wk[:, 1:2])
                    r01 = s2p.tile([P, 1], FP32, tag="r01")
                    nc.vector.tensor_mul(r01, wk[:, 0:1], rw1)
                    for ci, (v0, w) in enumerate(vchunks):
                        t = ttp.tile([P, VCHUNK], FP32, tag="t")
                        nc.vector.tensor_scalar_mul(
                            t[:, :w], E[:, 0, v0:v0 + w], r01
                        )
                        nc.vector.tensor_add(t[:, :w], t[:, :w],
                                             E[:, 1, v0:v0 + w])
                        oc = ocp.tile([P, VCHUNK], FP32, tag="oc")
                        act_chain(nc.scalar.activation(
                            out=oc[:, :w],
                            in_=t[:, :w],
                            func=ln_func,
                            scale=wk[:, 1:2],
                        ))
                        nc.sync.dma_start(
                            out=out[srow:srow + P, v0:v0 + w], in_=oc[:, :w]
                        )

                pending = None  # (i, E, Zg) awaiting stage 2
                for i in range(ST):
                    if i not in pj_tiles:
                        pj_tiles[i] = load_pj(i)
                    nxt = i + PJ_PRELOAD
                    if nxt < ST and nxt not in pj_tiles:
                        pj_tiles[nxt] = load_pj(nxt)
                    PJ = pj_tiles.pop(i)
                    E = ep.tile([P, KM, VS], e_dtype)
                    zpart = zpp.tile([P, KM, NVC], FP32)
                    emit_exps(i, 0, E, zpart, PJ)
                    if pending is not None:
                        emit_stage2(*pending)
                        pending = None
                    for k in range(1, KM):
                        emit_exps(i, k, E, zpart, PJ)
                    zloc = s2p.tile([P, KM], FP32, tag="zloc")
                    for k in range(KM):
                        nc.vector.reduce_sum(
                            out=zloc[:, k:k + 1],
                            in_=zpart[:, k, :],
                            axis=mybir.AxisListType.X,
                        )
                    # remove pad-column contribution (exp(0)=1 per pad col)
                    nc.vector.tensor_scalar_sub(zloc, zloc, corr_sb)

                    cin = ccp.tile([P, KM], FP32, tag="cin")
                    cout = ccp.tile([P, KM], FP32, tag="cout",
                                    addr_space=cc_addr)
                    nc.sync.dma_start(out=cin, in_=zloc)
                    if use_collectives:
                        nc.gpsimd.collective_compute(
                            "AllReduce",
                            mybir.AluOpType.add,
                            replica_groups=RG,
                            ins=[cin.opt()],
                            outs=[cout.opt()],
                        )
                    else:
                        nc.sync.dma_start(out=cout, in_=cin)
                    Zg = s2p.tile([P, KM], FP32, tag="zg")
                    nc.sync.dma_start(out=Zg, in_=cout)
                    pending = (i, E, Zg)
                emit_stage2(*pending)

    with tile.TileContext(nc) as tc:
        for _ in range(reps):
            emit_once(tc)

    nc.compile()
    return nc


def prep_inputs(hidden, weight_matrix, w_proj, w_gate, n_cores=8):
    """Host-side shard/transpose/cast. Returns (in_maps, VS, pad)."""
    bf16 = ml_dtypes.bfloat16
    B, S, D = hidden.shape
    V = weight_matrix.shape[0]
    VS = _ceil_div(V, n_cores)
    VP = VS * n_cores
    pad = VP - V

    hiddenT = np.ascontiguousarray(
        np.asarray(hidden, dtype=np.float32).reshape(S, D).T
    ).astype(bf16)
    w_projT = np.ascontiguousarray(
        np.asarray(w_proj, dtype=np.float32).T
    ).astype(bf16)
    w_gateT = np.ascontiguousarray(
        np.asarray(w_gate, dtype=np.float32).T
    ).astype(bf16)

    wmat = np.asarray(weight_matrix, dtype=np.float32)
    SSH = S // n_cores
    in_maps = []
    for c in range(n_cores):
        lo = c * VS
        hi = min(lo + VS, V)
        shard = np.zeros((VS, D), dtype=np.float32)
        shard[: hi - lo] = wmat[lo:hi]
        wt_c = np.ascontiguousarray(shard.T).astype(bf16)
        npad = VS - (hi - lo)
        corr_c = np.full((P, 1), float(npad), dtype=np.float32)
        in_maps.append(
            {
                "hiddenT": hiddenT,
                "hiddenTs": np.ascontiguousarray(
                    hiddenT[:, c * SSH:(c + 1) * SSH]
                ),
                "w_projT": w_projT,
                "w_gateT": w_gateT,
                "wt": wt_c,
                "corr": corr_c,
            }
        )
    return in_maps, VS, pad


_PROGRAM_CACHE = {}


def kernel(hidden, weight_matrix, w_proj, w_gate):
    import time

    n_cores = 8
    B, S, D = hidden.shape
    V = weight_matrix.shape[0]
    KM = w_gate.shape[0]
    in_maps, VS, pad = prep_inputs(hidden, weight_matrix, w_proj, w_gate, n_cores)

    key = (n_cores, S, D, VS, KM)
    if key not in _PROGRAM_CACHE:
        _PROGRAM_CACHE[key] = build_program(n_cores, S, D, VS, KM)
    nc = _PROGRAM_CACHE[key]

    # The axon terminal occasionally reports a transient
    # NRT_EXEC_UNIT_UNRECOVERABLE right after another process released the
    # devices; one retry after a pause usually succeeds.
    last_err = None
    for attempt in range(2):
        try:
            res = run_bass_kernel_spmd(nc, in_maps, core_ids=list(range(n_cores)))
            break
        except Exception as e:  # noqa: BLE001
            last_err = e
            time.sleep(15)
    else:
        raise last_err

    full = np.empty((S, VS * n_cores), dtype=np.float32)
    for c in range(n_cores):
        full[:, c * VS:(c + 1) * VS] = res.results[c]["out"]
    return full[:, :V].reshape(B, S, V)



# revision 2
# speedup vs baseline: 2.3889x; 2.3889x over previous
"""Mixture-of-Softmax loss kernel for 8 Trainium2 NeuronCores.

out[s,v] = logsumexp_k( log_softmax_v(logits[s,k,v]) + log pi[s,k] )
         = log( sum_k pi[s,k] * exp(logits[s,k,v]) / Z[s,k] )

Sharding: vocab dimension of weight_matrix split across 8 cores. Per-core
logical shard width VS=6283 (V=50257 -> 8*6283=50264), padded on-chip to
VSP=6288 (= 12*512 + 144, multiple of 16 for fp8 DoubleRow APs) with zero
weight columns. Pad columns contribute exp(0)=1 to the local sum-of-exp and
are subtracted via the per-core `corr` input, then dropped on gather.

The big [S,K,V] logits matmul runs in fp8-e4m3 with perf_mode=DoubleRow
(2 contraction rows per PE pass). weight_matrix is scaled by 256 on the host
before the fp8 cast (its std is 0.02, below e4m3's min normal) and the Exp
activation un-scales with its free affine (scale=1/256). projT is cast to
fp8 on device after phase 0. Logits are small (|l| < ~4) so no max
subtraction is needed for a stable sum-of-exp in fp32.

Per core, per 128-token s-tile:
  PE   : logits[k] = projT[k]^T @ WT    (fp8 DoubleRow, fp32 PSUM)
  ACT  : E = exp(logits/256) (fp16) in 2048-wide reads across 4 PSUM banks,
         accum_out = per-group sums
  CC   : AllReduce(add) of local [128,2] sum-of-exp -> global Z
  DVE  : w_k = pi_k / Z_k ;  t = E0*(w0/w1) + E1   (one fused fp16 pass)
  ACT  : out = Ln(t * w1)  (one 6288-wide fp16 pass)
"""

import math
import os
import sys

import numpy as np

for _p in ("/opt/trn_rl_repo", "/opt/trn_rl_repo/concourse"):
    if os.path.isdir(_p) and _p not in sys.path:
        sys.path.insert(0, _p)

import ml_dtypes

import concourse.bacc as bacc
import concourse.hw_specs as hw_specs
import concourse.tile as tile
from concourse import mybir
from concourse.bass_utils import run_bass_kernel_spmd

# --- Activation-table patch -------------------------------------------------
# This kernel interleaves Exp (sum-of-exp pass) and Ln (output pass) on the
# scalar engine. The default table chooser assigns Exp -> "exp_and_others"
# and Ln -> "natural_log", causing a ~2.7us ACT_TABLE_LOAD on every switch.
# The "natural_log_exp_and_others" set contains BOTH functions; hide Exp/Ln
# from every other set so the chooser must use the combined set, making the
# table resident for the whole kernel.
_orig_get_activation_tables = hw_specs.get_activation_tables


def _patched_get_activation_tables(module_arch):
    tabs = _orig_get_activation_tables(module_arch)
    E = mybir.ActivationFunctionType.Exp
    L = mybir.ActivationFunctionType.Ln
    out = {}
    for name, funcs in tabs.items():
        if name != "natural_log_exp_and_others" and (E in funcs or L in funcs):
            funcs = funcs - {E, L}
        out[name] = funcs
    return out


bacc.get_activation_tables = _patched_get_activation_tables
# ---------------------------------------------------------------------------

BF16 = mybir.dt.bfloat16
FP16 = mybir.dt.float16
FP32 = mybir.dt.float32
FP8 = mybir.dt.float8e4
P = 128  # partitions
W_SCALE = 256.0  # host-side weight_matrix scale before fp8 cast


def _ceil_div(a, b):
    return (a + b - 1) // b


def build_program(n_cores=8, S=2048, D=1024, VSP=6288, KM=2, e_dtype=FP16,
                  use_collectives=True, reps=1, ln_func=None):
    """Build the SPMD Bass program (same program on all cores).

    Inputs (per core):
      hiddenT  [D, S]   bf16   (same on all cores)
      hiddenTs [D, S/n] bf16   (this core's token slice)
      w_projT  [D, KM*D] bf16  (same on all cores)
      w_gateT  [D, KM]  bf16   (same on all cores)
      wt       [D, VSP] fp8e4  (core's vocab shard of weight_matrix^T * 256)
      corr     [P, 1]   f32    (number of zero-pad columns in this shard)
    Output (per core):
      out      [S, VSP] fp16
    """
    DC = D // P           # contraction chunks (128 rows each)
    NDP = DC // 2         # DoubleRow pairs (256 rows each)
    ST = S // P           # token tiles
    J = KM * D
    JT = J // P           # projT row tiles
    DR = mybir.MatmulPerfMode.DoubleRow
    # vocab groups: 4 PSUM banks (2048 f32) each, last group 144 wide
    groups = []
    v0 = 0
    while v0 < VSP:
        gw = min(2048, VSP - v0)
        groups.append((v0, gw))
        v0 += gw
    NG = len(groups)
    RG = [list(range(n_cores))]
    if ln_func is None:
        ln_func = mybir.ActivationFunctionType.Ln

    nc = bacc.Bacc(
        "TRN2",
        target_bir_lowering=False,
        debug=False,
        num_devices=n_cores,
    )

    hiddenT = nc.dram_tensor("hiddenT", [D, S], BF16, kind="ExternalInput").ap()
    hiddenTs = nc.dram_tensor(
        "hiddenTs", [D, S // n_cores], BF16, kind="ExternalInput"
    ).ap()
    w_projT = nc.dram_tensor("w_projT", [D, J], BF16, kind="ExternalInput").ap()
    w_gateT = nc.dram_tensor("w_gateT", [D, KM], BF16, kind="ExternalInput").ap()
    wt = nc.dram_tensor("wt", [D, VSP], FP8, kind="ExternalInput").ap()
    corr = nc.dram_tensor("corr", [P, 1], FP32, kind="ExternalInput").ap()
    out = nc.dram_tensor("out", [S, VSP], FP16, kind="ExternalOutput").ap()

    ht_r = hiddenT.rearrange("(c p) s -> c p s", p=P)
    hts_r = hiddenTs.rearrange("(c p) s -> c p s", p=P)
    wp_r = w_projT.rearrange("(c p) j -> c p j", p=P)
    wg_r = w_gateT.rearrange("(c p) k -> c p k", p=P)
    wt_r = wt.rearrange("(c p) v -> c p v", p=P)

    def emit_once(tc):
        with (
            tc.tile_pool(name="singles", bufs=1) as singles,
            tc.tile_pool(name="gates", bufs=ST) as gates,
            tc.tile_pool(name="dram", bufs=1, space="DRAM") as dpool,
            tc.tile_pool(name="pj", bufs=3) as pjp,
        ):
            PJ_PRELOAD = 3

            def load_pj(i):
                srow = i * P
                ci = srow // SSH
                soff = srow - ci * SSH
                PJ = pjp.tile([P, JT, P], FP8, tag="PJ", name=f"PJ_{i}")
                nc.sync.dma_start(
                    out=PJ,
                    in_=proj_ag[ci][:, :, soff:soff + P].rearrange(
                        "t p s -> p t s"
                    ),
                )
                return PJ

            # Resident fp8 vocab-shard weights, one tile per 2048-wide group
            # so the first matmuls only wait on their own slice of the load.
            WTs = []
            for gi, (v0, gw) in enumerate(groups):
                wt_tile = singles.tile([P, DC, gw], FP8, tag=f"wt{gi}",
                                       name=f"WT_{gi}")
                for c in range(DC):
                    nc.sync.dma_start(out=wt_tile[:, c, :],
                                      in_=wt_r[c][:, v0:v0 + gw])
                WTs.append(wt_tile)
            corr_sb = singles.tile([P, 1], FP32)
            nc.sync.dma_start(out=corr_sb, in_=corr)

            # Phase 0 is sharded over cores: each core computes projT for
            # S/n_cores tokens, then an AllGather replicates the full projT
            # (in fp8 -- the main loop consumes fp8 anyway).
            SSH = S // n_cores  # tokens per core in phase 0
            assert SSH % P == 0 or n_cores == 1
            proj_in = dpool.tile([JT, P, SSH], FP8, name="proj_in")
            cc_addr = "Shared" if n_cores > 4 else "Local"
            proj_ag = dpool.tile([n_cores, JT, P, SSH], FP8, name="proj_ag",
                                 addr_space=cc_addr)
            ge_tiles = []
            rse_tiles = []

            # ACT-order chain: order-only edges keep the scalar engine's
            # instruction stream in emission order so Exp/Ln stay batched.
            last_act = [None]

            def act_chain(inst):
                if last_act[0] is not None:
                    tile.add_dep_helper(inst.ins, last_act[0].ins, sync=False,
                                        reason="act table batching")
                last_act[0] = inst
                return inst

            # ---------------- Phase 0: projT = (hidden @ w_proj^T)^T, gate ----
            with (
                tc.tile_pool(name="ph0", bufs=1) as ph0,
                tc.tile_pool(name="ph0ps", bufs=4, space="PSUM") as ps0,
                tc.tile_pool(name="ph0gps", bufs=2, space="PSUM") as gps0,
                tc.tile_pool(name="ph0st", bufs=4) as stg,
            ):
                HT = ph0.tile([P, DC, S], BF16)
                HTS = ph0.tile([P, DC, SSH], BF16)
                WP = ph0.tile([P, DC, J], BF16)
                WG = ph0.tile([P, DC, KM], BF16)
                for c in range(DC):
                    nc.sync.dma_start(out=HTS[:, c, :], in_=hts_r[c])
                    nc.sync.dma_start(out=WP[:, c, :], in_=wp_r[c])
                    nc.sync.dma_start(out=WG[:, c, :], in_=wg_r[c])
                    nc.sync.dma_start(out=HT[:, c, :], in_=ht_r[c])

                # projT[j, s] = sum_d w_projT[d, j] * hiddenT[d, s], for
                # this core's S/n_cores token slice; AllGather replicates.
                pj_tiles = {}
                PSC = min(512, SSH)
                for t in range(JT):
                    for s0 in range(0, SSH, PSC):
                        sw = min(PSC, SSH - s0)
                        psum = ps0.tile([P, PSC], FP32, tag="mm")
                        for d in range(DC):
                            nc.tensor.matmul(
                                psum[:, :sw],
                                lhsT=WP[:, d, t * P:(t + 1) * P],
                                rhs=HTS[:, d, s0:s0 + sw],
                                start=(d == 0),
                                stop=(d == DC - 1),
                            )
                        st = stg.tile([P, PSC], FP8, tag="st")
                        nc.vector.tensor_copy(st[:, :sw], psum[:, :sw])
                        nc.sync.dma_start(out=proj_in[t, :, s0:s0 + sw],
                                          in_=st[:, :sw])
                if use_collectives:
                    nc.gpsimd.collective_compute(
                        "AllGather",
                        mybir.AluOpType.bypass,
                        replica_groups=RG,
                        ins=[proj_in.opt()],
                        outs=[proj_ag.opt()],
                    )
                else:
                    nc.sync.dma_start(out=proj_ag[0], in_=proj_in[:])
                # Prefetch the first main-loop lhsT slices now so their
                # DMAs aren't queued behind the rest of phase 0.
                for i in range(min(PJ_PRELOAD, ST)):
                    pj_tiles[i] = load_pj(i)

                # gate logits -> pi (unnormalized e, and 1/sum_e)
                for i in range(ST):
                    gp = gps0.tile([P, KM], FP32, tag="g")
                    for d in range(DC):
                        nc.tensor.matmul(
                            gp,
                            lhsT=HT[:, d, i * P:(i + 1) * P],
                            rhs=WG[:, d, :],
                            start=(d == 0),
                            stop=(d == DC - 1),
                        )
                    negm = gates.tile([P, 1], FP32, tag="negm")
                    nc.vector.reduce_max(
                        out=negm, in_=gp, axis=mybir.AxisListType.X, negate=True
                    )
                    ge = gates.tile([P, KM], FP32, tag="ge")
                    se = gates.tile([P, 1], FP32, tag="se")
                    act_chain(nc.scalar.activation(
                        out=ge, in_=gp, func=mybir.ActivationFunctionType.Exp,
                        bias=negm, accum_out=se,
                    ))
                    rse = gates.tile([P, 1], FP32, tag="rse")
                    nc.vector.reciprocal(rse, se)
                    ge_tiles.append(ge)
                    rse_tiles.append(rse)

            # ---------------- Main loop over token tiles ----------------------
            with (
                tc.tile_pool(name="ebuf", bufs=3) as ep,
                tc.tile_pool(name="zp", bufs=3) as zpp,
                tc.tile_pool(name="mmps", bufs=2, space="PSUM") as psm,
                tc.tile_pool(name="ocp", bufs=2) as ocp,
                tc.tile_pool(name="ttp", bufs=2) as ttp,
                tc.tile_pool(name="s2", bufs=3) as s2p,
                tc.tile_pool(name="cc", bufs=2 * ST, space="DRAM") as ccp,
            ):
                def emit_exps(i, k, E, zpart, PJ):
                    for g, (v0, gw) in enumerate(groups):
                        ps = psm.tile([P, 2048], FP32, tag="mm")
                        nchunks = _ceil_div(gw, 512)
                        for j in range(NDP):
                            lhsT = PJ[:, k * DC + 2 * j:k * DC + 2 * j + 2, :]
                            for c in range(nchunks):
                                cw = min(512, gw - c * 512)
                                nc.tensor.matmul(
                                    ps[:, c * 512:c * 512 + cw],
                                    lhsT=lhsT,
                                    rhs=WTs[g][:, 2 * j:2 * j + 2,
                                               c * 512:c * 512 + cw],
                                    start=(j == 0),
                                    stop=(j == NDP - 1),
                                    perf_mode=DR,
                                )
                        act_chain(nc.scalar.activation(
                            out=E[:, k, v0:v0 + gw],
                            in_=ps[:, :gw],
                            func=mybir.ActivationFunctionType.Exp,
                            scale=1.0 / W_SCALE,
                            accum_out=zpart[:, k, g:g + 1],
                        ))

                def emit_stage2(i, E, Zg):
                    srow = i * P
                    # w_k = pi_k / Z_k = ge_k * rse / Z_k
                    rz = s2p.tile([P, KM], FP32, tag="rz")
                    nc.vector.reciprocal(rz, Zg)
                    rzs = s2p.tile([P, KM], FP32, tag="rzs")
                    nc.vector.tensor_scalar_mul(rzs, rz, rse_tiles[i])
                    wk = s2p.tile([P, KM], FP32, tag="wk")
                    nc.vector.tensor_mul(wk, ge_tiles[i], rzs)
                    rw1 = s2p.tile([P, 1], FP32, tag="rw1")
                    nc.vector.reciprocal(rw1, wk[:, 1:2])
                    r01 = s2p.tile([P, 1], FP32, tag="r01")
                    nc.vector.tensor_mul(r01, wk[:, 0:1], rw1)
                    # t = E0 * (w0/w1) + E1, one fused DVE pass in fp16
                    t = ttp.tile([P, VSP], FP16, tag="t")
                    nc.vector.scalar_tensor_tensor(
                        out=t,
                        in0=E[:, 0, :],
                        scalar=r01,
                        in1=E[:, 1, :],
                        op0=mybir.AluOpType.mult,
                        op1=mybir.AluOpType.add,
                    )
                    oc = ocp.tile([P, VSP], FP16, tag="oc")
                    act_chain(nc.scalar.activation(
                        out=oc,
                        in_=t,
                        func=ln_func,
                        scale=wk[:, 1:2],
                    ))
                    nc.sync.dma_start(out=out[srow:srow + P, :], in_=oc)

                pending = []  # [(i, E, Zg)] awaiting stage 2 (depth 2)
                for i in range(ST):
                    if i not in pj_tiles:
                        pj_tiles[i] = load_pj(i)
                    nxt = i + PJ_PRELOAD
                    if nxt < ST and nxt not in pj_tiles:
                        pj_tiles[nxt] = load_pj(nxt)
                    PJ = pj_tiles.pop(i)
                    E = ep.tile([P, KM, VSP], e_dtype)
                    zpart = zpp.tile([P, KM, NG], FP32)
                    emit_exps(i, 0, E, zpart, PJ)
                    if len(pending) >= 2:
                        emit_stage2(*pending.pop(0))
                    for k in range(1, KM):
                        emit_exps(i, k, E, zpart, PJ)
                    zloc = s2p.tile([P, KM], FP32, tag="zloc")
                    for k in range(KM):
                        nc.vector.reduce_sum(
                            out=zloc[:, k:k + 1],
                            in_=zpart[:, k, :],
                            axis=mybir.AxisListType.X,
                        )
                    # remove pad-column contribution (exp(0)=1 per pad col)
                    nc.vector.tensor_scalar_sub(zloc, zloc, corr_sb)

                    cin = ccp.tile([P, KM], FP32, tag="cin")
                    cout = ccp.tile([P, KM], FP32, tag="cout",
                                    addr_space=cc_addr)
                    nc.sync.dma_start(out=cin, in_=zloc)
                    if use_collectives:
                        nc.gpsimd.collective_compute(
                            "AllReduce",
                            mybir.AluOpType.add,
                            replica_groups=RG,
                            ins=[cin.opt()],
                            outs=[cout.opt()],
                        )
                    else:
                        nc.sync.dma_start(out=cout, in_=cin)
                    Zg = s2p.tile([P, KM], FP32, tag="zg")
                    nc.sync.dma_start(out=Zg, in_=cout)
                    pending.append((i, E, Zg))
                while pending:
                    emit_stage2(*pending.pop(0))

    with tile.TileContext(nc) as tc:
        for _ in range(reps):
            emit_once(tc)

    nc.compile()
    return nc


def prep_inputs(hidden, weight_matrix, w_proj, w_gate, n_cores=8):
    """Host-side shard/transpose/cast. Returns (in_maps, VS, VSP)."""
    bf16 = ml_dtypes.bfloat16
    fp8 = ml_dtypes.float8_e4m3
    B, S, D = hidden.shape
    V = weight_matrix.shape[0]
    VS = _ceil_div(V, n_cores)       # logical shard width (6283)
    VSP = _ceil_div(VS, 16) * 16     # on-chip width, multiple of 16 (6288)

    hiddenT = np.ascontiguousarray(
        np.asarray(hidden, dtype=np.float32).reshape(S, D).T
    ).astype(bf16)
    w_projT = np.ascontiguousarray(
        np.asarray(w_proj, dtype=np.float32).T
    ).astype(bf16)
    w_gateT = np.ascontiguousarray(
        np.asarray(w_gate, dtype=np.float32).T
    ).astype(bf16)

    wmat = np.asarray(weight_matrix, dtype=np.float32)
    SSH = S // n_cores
    in_maps = []
    for c in range(n_cores):
        lo = c * VS
        hi = min(lo + VS, V)
        shard = np.zeros((VSP, D), dtype=np.float32)
        shard[: hi - lo] = wmat[lo:hi]
        wt_c = np.clip(
            np.ascontiguousarray(shard.T) * W_SCALE, -240.0, 240.0
        ).astype(fp8)
        npad = VSP - (hi - lo)
        corr_c = np.full((P, 1), float(npad), dtype=np.float32)
        in_maps.append(
            {
                "hiddenT": hiddenT,
                "hiddenTs": np.ascontiguousarray(
                    hiddenT[:, c * SSH:(c + 1) * SSH]
                ),
                "w_projT": w_projT,
                "w_gateT": w_gateT,
                "wt": wt_c,
                "corr": corr_c,
            }
        )
    return in_maps, VS, VSP


_PROGRAM_CACHE = {}


def kernel(hidden, weight_matrix, w_proj, w_gate):
    import time

    n_cores = 8
    B, S, D = hidden.shape
    V = weight_matrix.shape[0]
    KM = w_gate.shape[0]
    in_maps, VS, VSP = prep_inputs(hidden, weight_matrix, w_proj, w_gate,
                                   n_cores)

    key = (n_cores, S, D, VSP, KM)
    if key not in _PROGRAM_CACHE:
        _PROGRAM_CACHE[key] = build_program(n_cores, S, D, VSP, KM)
    nc = _PROGRAM_CACHE[key]

    # The axon terminal occasionally reports a transient
    # NRT_EXEC_UNIT_UNRECOVERABLE right after another process released the
    # devices; one retry after a pause usually succeeds.
    last_err = None
    for attempt in range(2):
        try:
            res = run_bass_kernel_spmd(nc, in_maps, core_ids=list(range(n_cores)))
            break
        except Exception as e:  # noqa: BLE001
            last_err = e
            time.sleep(15)
    else:
        raise last_err

    full = np.empty((S, VS * n_cores), dtype=np.float32)
    for c in range(n_cores):
        full[:, c * VS:(c + 1) * VS] = res.results[c]["out"][:, :VS]
    return full[:, :V].reshape(B, S, V)
